# revision 15
# baseline (speedup 1.0000x reference)
"""Trainium2 Bass kernel for nn_CGLayers (GElib-style CG message passing).

Strategy
--------
The only large data is the per-pair weights wrel{L}_{l}: 2 layers x
(32,32,tau_l,16) f32 = 503 MB total -> memory-bound stream.  Everything
else (CG tables, spherical harmonics Y, neighbor sums mp, cg_nl,
mixed_nl/rep) is tiny and depends on wrel only through the previous
layer's output, so it is computed on the host between two device
launches (one per layer).

Key algebraic fact: in cg_rel = cg_product(rep, sph) the sph channels
are all identical (reference repeats Y across C), so
  cg_rel[b,i,j,z,(q,c,d)] = A_q[b,i,j,z,c]           (d-independent)
  out[b,i,z,c'] = sum_{j,q,c} A_q[b,i,j,z,c] * (sum_d W[i,j,(q,c,d),c'])
The device therefore streams W with rows K=(j,q,c) and free axis
N=(d,c')=256 (contiguous 1KB blocks in the natural wrel layout), doing
PSUM-accumulated matmuls with the host-built A as the stationary
operand, then reduces over d (a 4-step halving add on the 256-wide
PSUM result).  Sharding: atom axis i (adim0) split 4-per-core across
8 cores; no collectives needed.
"""

import math
import sys

import numpy as np

sys.path.insert(0, "/opt/trn_rl_repo")

MAXL = 2
C = 16
A = 32
B = 2
LAYERS = 2
CUT = 0.5
NCORES = 8
IL = A // NCORES  # atoms per core

# couplings (l1,l2) per output l, in reference loop order (l1 outer, l2 inner)
QL = [
    [(0, 0), (1, 1), (2, 2)],
    [(0, 1), (1, 0), (1, 1), (1, 2), (2, 1), (2, 2)],
    [(0, 2), (1, 1), (1, 2), (2, 0), (2, 1), (2, 2)],
]
NQ = [len(q) for q in QL]          # 3, 6, 6
ZL = [1, 3, 5]                     # 2l+1
ML = [B * z for z in ZL]           # matmul M per output l: 2, 6, 10
ROWS = [A * nq * C for nq in NQ]   # K rows per i: 1536, 3072, 3072
NCH = [r // 96 for r in ROWS]      # 96-row chunks per i: 16, 32, 32


def _cg_coeff(l1, l2, l, m1, m2, m):
    if m1 + m2 != m or l < abs(l1 - l2) or l > l1 + l2:
        return 0.0
    f = math.factorial
    pre = math.sqrt((2 * l + 1) * f(l + l1 - l2) * f(l - l1 + l2) * f(l1 + l2 - l) / f(l1 + l2 + l + 1))
    pre *= math.sqrt(f(l + m) * f(l - m) * f(l1 + m1) * f(l1 - m1) * f(l2 + m2) * f(l2 - m2))
    s = 0.0
    for k in range(0, l1 + l2 + 1):
        d = (k, l1 + l2 - l - k, l1 - m1 - k, l2 + m2 - k, l - l2 + m1 + k, l - l1 - m2 + k)
        if min(d) < 0:
            continue
        s += (-1.0) ** k / (f(d[0]) * f(d[1]) * f(d[2]) * f(d[3]) * f(d[4]) * f(d[5]))
    return pre * s


def _cg_matrix(l1, l2, l):
    M = np.zeros((2 * l1 + 1, 2 * l2 + 1, 2 * l + 1), dtype=np.float32)
    for m1 in range(-l1, l1 + 1):
        for m2 in range(-l2, l2 + 1):
            m = m1 + m2
            if -l <= m <= l:
                M[m1 + l1, m2 + l2, m + l] = _cg_coeff(l1, l2, l, m1, m2, m)
    return M


_CG = {}
for _l1 in range(MAXL + 1):
    for _l2 in range(MAXL + 1):
        for _l in range(abs(_l1 - _l2), min(_l1 + _l2, MAXL) + 1):
            _CG[(_l1, _l2, _l)] = _cg_matrix(_l1, _l2, _l)


def _sph_np(rel_pos):
    """Real spherical harmonics of normalized rel vectors -> [(B,A,A,2l+1)]."""
    eps = np.float32(1e-6)
    r = np.sqrt(np.sum(rel_pos * rel_pos, axis=-1, keepdims=True) + eps)
    u = rel_pos / r
    x, y, z = u[..., 0], u[..., 1], u[..., 2]
    Y0 = np.float32(0.28209479) * np.ones_like(x)[..., None]
    Y1 = np.float32(0.48860251) * np.stack([y, z, x], axis=-1)
    Y2 = np.stack(
        [
            np.float32(1.09254843) * x * y,
            np.float32(1.09254843) * y * z,
            np.float32(0.31539157) * (np.float32(3.0) * z * z - np.float32(1.0)),
            np.float32(1.09254843) * x * z,
            np.float32(0.54627422) * (x * x - y * y),
        ],
        axis=-1,
    )
    return [Y0, Y1, Y2]


def _cg_product_np(parts1, parts2):
    out = [[] for _ in range(MAXL + 1)]
    for l1, p1 in enumerate(parts1):
        for l2, p2 in enumerate(parts2):
            for l in range(abs(l1 - l2), min(l1 + l2, MAXL) + 1):
                t = np.einsum("xyz,...xc,...yd->...zcd", _CG[(l1, l2, l)], p1, p2)
                out[l].append(t.reshape(t.shape[:-2] + (t.shape[-2] * t.shape[-1],)))
    return [np.concatenate(o, axis=-1) for o in out]


def _scalars_np(parts):
    b, a = parts[0].shape[0], parts[0].shape[1]
    s0 = parts[0].reshape(b, a, C)
    sn = [np.einsum("baomc,baomd->bacd", p, p).reshape(b, a, C * C) for p in parts]
    return np.concatenate([s0] + sn, axis=-1)


_PROG = None
_LAST_EXEC_NS = []


def _build_program():
    """One SPMD Bass program: per core, 4 atoms x 3 output-l weight streams."""
    global _PROG
    if _PROG is not None:
        return _PROG
    import concourse.bass as bass
    import concourse.mybir as mybir
    from concourse.tile import TileContext

    f32 = mybir.dt.float32
    nc = bass.Bass()
    wa_in, o_out = [], []
    for l in range(3):
        wa_in.append(
            nc.dram_tensor(f"wa{l}", [IL, 96, NCH[l] * (256 + ML[l])], f32, kind="ExternalInput")
        )
        o_out.append(nc.dram_tensor(f"o{l}", [ML[l], IL * C], f32, kind="ExternalOutput"))

    with TileContext(nc) as tc:
        with (
            tc.tile_pool(name="z", bufs=1) as zp,
            tc.tile_pool(name="w", bufs=2) as wp,
            tc.tile_pool(name="ps", bufs=2, space=bass.MemorySpace.PSUM) as pp,
            tc.tile_pool(name="s", bufs=2) as sp,
        ):
            # zero tile: feeds a dummy first matmul per group so the psum
            # buffer-reuse wait lands on a PE op with no other deps
            # (Matmult supports only a single sync wait on TRN2)
            z = zp.tile([16, 256], f32)
            nc.gpsimd.memset(z[:], 0.0)
            # per-l output staging: all (i,l) results land here via DVE,
            # then 3 contiguous DMAs at the end (keeps every inst <=1 wait)
            ob = [
                zp.tile([ML[l], IL * C], f32, tag=f"ob{l}", name=f"ob{l}")
                for l in range(3)
            ]
            for i in range(IL):
                for l in range(3):
                    m = ML[l]
                    woff = NCH[l] * 256
                    wt = wp.tile([96, NCH[l] * (256 + m)], f32, tag=f"w{l}")
                    nc.gpsimd.dma_start(wt[:], wa_in[l][i])
                    ps = pp.tile([m, 256], f32, tag=f"ps{l}")
                    nc.tensor.matmul(ps[:], z[:, 0:m], z[:, 0:256], start=True, stop=False)
                    for k in range(NCH[l]):
                        nc.tensor.matmul(
                            ps[:],
                            wt[:, woff + k * m: woff + (k + 1) * m],
                            wt[:, k * 256:(k + 1) * 256],
                            start=False,
                            stop=(k == NCH[l] - 1),
                        )
                    # d-sum: free axis is (d,c') with d outer; halve d 4 times
                    st = sp.tile([m, 256], f32, tag=f"s{l}")
                    nc.vector.tensor_copy(st[:], ps[:])
                    nc.vector.tensor_add(st[:, 0:128], st[:, 0:128], st[:, 128:256])
                    nc.vector.tensor_add(st[:, 0:64], st[:, 0:64], st[:, 64:128])
                    nc.vector.tensor_add(st[:, 0:32], st[:, 0:32], st[:, 32:64])
                    nc.vector.tensor_add(
                        ob[l][:, i * C:(i + 1) * C], st[:, 0:16], st[:, 16:32]
                    )
            for l in range(3):
                nc.gpsimd.dma_start(o_out[l][:], ob[l][:])

    nc.finalize()
    # Walrus accepts at most 1 sync wait per engine instruction (2 on
    # InstEventSemaphore); Tile/bacc leave 2 on some DMACopies.  Move
    # excess waits onto EventSemaphore instructions inserted just before.
    for f in nc.m.functions:
        for blk in f.blocks:
            idx = 0
            while idx < len(blk.instructions):
                inst = blk.instructions[idx]
                si = getattr(inst, "sync_info", None)
                if (
                    si is not None
                    and si.on_wait
                    and len(si.on_wait) > 1
                    and not isinstance(inst, mybir.InstEventSemaphore)
                ):
                    moved, keep = si.on_wait[:-1], si.on_wait[-1:]
                    for wi in range(0, len(moved), 2):
                        ev = mybir.InstEventSemaphore(name=nc.get_next_instruction_name())
                        ev.engine = inst.engine
                        ev.sync_info = mybir.SyncInfo(
                            on_wait=list(moved[wi:wi + 2]), on_update=[]
                        )
                        nc.register_instruction(ev)
                        blk.instructions.insert(idx, ev)
                        idx += 1
                    si.on_wait = keep
                idx += 1
    _PROG = nc
    return nc


def _pack_w(wrel_l, l):
    """(A,A,tau,16) -> per-i SBUF image (A, 96, NCH*256), rows (j,q,c)."""
    w2 = np.ascontiguousarray(wrel_l, dtype=np.float32).reshape(A, NCH[l], 96, 256)
    return np.ascontiguousarray(w2.transpose(0, 2, 1, 3)).reshape(A, 96, NCH[l] * 256)


def _build_A(rep, Y, l):
    """Host lhsT: (A_i, 96, NCH*M) with rows (j,q,c), cols (b,z)."""
    aq = []
    for (l1, l2) in QL[l]:
        # A_q[b,i,j,z,c] = sum_{x,y} CG[x,y,z] rep[l1][b,i,x,c] Y[l2][b,i,j,y]
        aq.append(np.einsum("xyz,bixc,bijy->bijzc", _CG[(l1, l2, l)], rep[l1], Y[l2]))
    al = np.stack(aq, axis=3)  # (B,i,j,nq,Z,C)
    # -> (i, j, nq, C, B, Z) -> rows (j,q,c) x cols (b,z)
    al = np.ascontiguousarray(al.transpose(1, 2, 3, 5, 0, 4), dtype=np.float32)
    al = al.reshape(A, ROWS[l], ML[l])
    al = al.reshape(A, NCH[l], 96, ML[l]).transpose(0, 2, 1, 3)
    return np.ascontiguousarray(al).reshape(A, 96, NCH[l] * ML[l])


def kernel(**inputs):
    from concourse.bass_utils import run_bass_kernel_spmd

    _LAST_EXEC_NS.clear()

    v0 = np.asarray(inputs["v0"], dtype=np.float32)
    v1 = np.asarray(inputs["v1"], dtype=np.float32)
    v2 = np.asarray(inputs["v2"], dtype=np.float32)
    rel_pos = np.asarray(inputs["rel_pos"], dtype=np.float32)
    norms = np.asarray(inputs["norms"], dtype=np.float32)
    wnl = [[np.asarray(inputs[f"wnl{L}_{l}"], dtype=np.float32) for l in range(3)] for L in range(2)]
    wrel = [[np.asarray(inputs[f"wrel{L}_{l}"], dtype=np.float32) for l in range(3)] for L in range(2)]

    Y = _sph_np(rel_pos)
    conn = (norms < CUT).astype(np.float32)
    w_packs = [[_pack_w(wrel[L][l], l) for l in range(3)] for L in range(2)]

    nc = _build_program()
    parts = [v0, v1, v2]
    outs = []
    for L in range(LAYERS):
        mp = [np.einsum("bij,bjomc->biomc", conn, p) for p in parts]
        cg_nl = _cg_product_np(mp, mp)
        mixed_nl = [np.einsum("baomt,aotc->baomc", cg_nl[l], wnl[L][l]) for l in range(3)]
        rep = [mixed_nl[l][:, :, 0] for l in range(3)]  # (B,A,2l+1,C)
        a_packs = [_build_A(rep, Y, l) for l in range(3)]

        wa = [
            np.ascontiguousarray(np.concatenate([w_packs[L][l], a_packs[l]], axis=2))
            for l in range(3)
        ]
        in_maps = []
        for r in range(NCORES):
            sl = slice(r * IL, (r + 1) * IL)
            in_maps.append({f"wa{l}": np.ascontiguousarray(wa[l][sl]) for l in range(3)})

        res = run_bass_kernel_spmd(nc, in_maps, core_ids=list(range(NCORES)))
        if getattr(res, "exec_time_ns", None) is not None:
            _LAST_EXEC_NS.append(res.exec_time_ns)

        parts = []
        for l in range(3):
            po = np.zeros((B, A, 1, ZL[l], C), dtype=np.float32)
            for r in range(NCORES):
                o = res.results[r][f"o{l}"]  # (B*Z, IL*C)
                o = o.reshape(B, ZL[l], IL, C).transpose(0, 2, 1, 3)
                po[:, r * IL:(r + 1) * IL, 0] = o
            parts.append(po)
        outs.append(_scalars_np(parts))

    return np.concatenate(outs, axis=0)


# revision 16
# speedup vs baseline: 1.4378x; 1.4378x over previous
"""Trainium2 Bass kernel for nn_CGLayers (GElib-style CG message passing).

Strategy
--------
The only large data is the per-pair weights wrel{L}_{l}: 2 layers x
(32,32,tau_l,16) f32 = 503 MB total -> memory-bound stream.  Everything
else (CG tables, spherical harmonics Y, neighbor sums mp, cg_nl,
mixed_nl/rep) is tiny and depends on wrel only through the previous
layer's output, so it is computed on the host between two device
launches (one per layer).

Key algebraic fact: in cg_rel = cg_product(rep, sph) the sph channels
are all identical (reference repeats Y across C), so
  cg_rel[b,i,j,z,(q,c,d)] = A_q[b,i,j,z,c]           (d-independent)
  out[b,i,z,c'] = sum_{j,q,c} A_q[b,i,j,z,c] * (sum_d W[i,j,(q,c,d),c'])
The device therefore streams W with rows K=(j,q,c) and free axis
N=(d,c')=256 (contiguous 1KB blocks in the natural wrel layout), doing
PSUM-accumulated matmuls with the host-built A as the stationary
operand, then reduces over d (a 4-step halving add on the 256-wide
PSUM result).  Sharding: atom axis i (adim0) split 4-per-core across
8 cores; no collectives needed.
"""

import math
import sys

import numpy as np

sys.path.insert(0, "/opt/trn_rl_repo")

MAXL = 2
C = 16
A = 32
B = 2
LAYERS = 2
CUT = 0.5
NCORES = 8
IL = A // NCORES  # atoms per core

# couplings (l1,l2) per output l, in reference loop order (l1 outer, l2 inner)
QL = [
    [(0, 0), (1, 1), (2, 2)],
    [(0, 1), (1, 0), (1, 1), (1, 2), (2, 1), (2, 2)],
    [(0, 2), (1, 1), (1, 2), (2, 0), (2, 1), (2, 2)],
]
NQ = [len(q) for q in QL]          # 3, 6, 6
ZL = [1, 3, 5]                     # 2l+1
ML = [B * z for z in ZL]           # matmul M per output l: 2, 6, 10
ROWS = [A * nq * C for nq in NQ]   # K rows per i: 1536, 3072, 3072
NCH = [r // 128 for r in ROWS]     # 128-row chunks per i: 12, 24, 24


def _cg_coeff(l1, l2, l, m1, m2, m):
    if m1 + m2 != m or l < abs(l1 - l2) or l > l1 + l2:
        return 0.0
    f = math.factorial
    pre = math.sqrt((2 * l + 1) * f(l + l1 - l2) * f(l - l1 + l2) * f(l1 + l2 - l) / f(l1 + l2 + l + 1))
    pre *= math.sqrt(f(l + m) * f(l - m) * f(l1 + m1) * f(l1 - m1) * f(l2 + m2) * f(l2 - m2))
    s = 0.0
    for k in range(0, l1 + l2 + 1):
        d = (k, l1 + l2 - l - k, l1 - m1 - k, l2 + m2 - k, l - l2 + m1 + k, l - l1 - m2 + k)
        if min(d) < 0:
            continue
        s += (-1.0) ** k / (f(d[0]) * f(d[1]) * f(d[2]) * f(d[3]) * f(d[4]) * f(d[5]))
    return pre * s


def _cg_matrix(l1, l2, l):
    M = np.zeros((2 * l1 + 1, 2 * l2 + 1, 2 * l + 1), dtype=np.float32)
    for m1 in range(-l1, l1 + 1):
        for m2 in range(-l2, l2 + 1):
            m = m1 + m2
            if -l <= m <= l:
                M[m1 + l1, m2 + l2, m + l] = _cg_coeff(l1, l2, l, m1, m2, m)
    return M


_CG = {}
for _l1 in range(MAXL + 1):
    for _l2 in range(MAXL + 1):
        for _l in range(abs(_l1 - _l2), min(_l1 + _l2, MAXL) + 1):
            _CG[(_l1, _l2, _l)] = _cg_matrix(_l1, _l2, _l)


def _sph_np(rel_pos):
    """Real spherical harmonics of normalized rel vectors -> [(B,A,A,2l+1)]."""
    eps = np.float32(1e-6)
    r = np.sqrt(np.sum(rel_pos * rel_pos, axis=-1, keepdims=True) + eps)
    u = rel_pos / r
    x, y, z = u[..., 0], u[..., 1], u[..., 2]
    Y0 = np.float32(0.28209479) * np.ones_like(x)[..., None]
    Y1 = np.float32(0.48860251) * np.stack([y, z, x], axis=-1)
    Y2 = np.stack(
        [
            np.float32(1.09254843) * x * y,
            np.float32(1.09254843) * y * z,
            np.float32(0.31539157) * (np.float32(3.0) * z * z - np.float32(1.0)),
            np.float32(1.09254843) * x * z,
            np.float32(0.54627422) * (x * x - y * y),
        ],
        axis=-1,
    )
    return [Y0, Y1, Y2]


def _cg_product_np(parts1, parts2):
    out = [[] for _ in range(MAXL + 1)]
    for l1, p1 in enumerate(parts1):
        for l2, p2 in enumerate(parts2):
            for l in range(abs(l1 - l2), min(l1 + l2, MAXL) + 1):
                t = np.einsum("xyz,...xc,...yd->...zcd", _CG[(l1, l2, l)], p1, p2)
                out[l].append(t.reshape(t.shape[:-2] + (t.shape[-2] * t.shape[-1],)))
    return [np.concatenate(o, axis=-1) for o in out]


def _scalars_np(parts):
    b, a = parts[0].shape[0], parts[0].shape[1]
    s0 = parts[0].reshape(b, a, C)
    sn = [np.einsum("baomc,baomd->bacd", p, p).reshape(b, a, C * C) for p in parts]
    return np.concatenate([s0] + sn, axis=-1)


_PROG = None
_LAST_EXEC_NS = []


def _build_program():
    """One SPMD Bass program: per core, 4 atoms x 3 output-l weight streams."""
    global _PROG
    if _PROG is not None:
        return _PROG
    import concourse.bass as bass
    import concourse.mybir as mybir
    from concourse.tile import TileContext

    f32 = mybir.dt.float32
    nc = bass.Bass()
    wa_in, o_out = [], []
    for l in range(3):
        wa_in.append(
            nc.dram_tensor(f"wa{l}", [IL, 128, NCH[l] * (256 + ML[l])], f32, kind="ExternalInput")
        )
        o_out.append(nc.dram_tensor(f"o{l}", [ML[l], IL * C], f32, kind="ExternalOutput"))

    with TileContext(nc) as tc:
        with (
            tc.tile_pool(name="z", bufs=1) as zp,
            tc.tile_pool(name="w", bufs=2) as wp,
            tc.tile_pool(name="ps", bufs=2, space=bass.MemorySpace.PSUM) as pp,
            tc.tile_pool(name="s", bufs=2) as sp,
        ):
            # zero tile: feeds a dummy first matmul per group so the psum
            # buffer-reuse wait lands on a PE op with no other deps
            # (Matmult supports only a single sync wait on TRN2)
            z = zp.tile([16, 256], f32)
            nc.gpsimd.memset(z[:], 0.0)
            # per-l output staging: all (i,l) results land here via DVE,
            # then 3 contiguous DMAs at the end (keeps every inst <=1 wait)
            ob = [
                zp.tile([ML[l], IL * C], f32, tag=f"ob{l}", name=f"ob{l}")
                for l in range(3)
            ]
            for i in range(IL):
                for l in range(3):
                    m = ML[l]
                    woff = NCH[l] * 256
                    wt = wp.tile([128, NCH[l] * (256 + m)], f32, tag=f"w{l}")
                    nc.gpsimd.dma_start(wt[:], wa_in[l][i])
                    ps = pp.tile([m, 256], f32, tag=f"ps{l}")
                    nc.tensor.matmul(ps[:], z[:, 0:m], z[:, 0:256], start=True, stop=False)
                    for k in range(NCH[l]):
                        nc.tensor.matmul(
                            ps[:],
                            wt[:, woff + k * m: woff + (k + 1) * m],
                            wt[:, k * 256:(k + 1) * 256],
                            start=False,
                            stop=(k == NCH[l] - 1),
                        )
                    # d-sum: free axis is (d,c') with d outer; halve d 4 times
                    st = sp.tile([m, 256], f32, tag=f"s{l}")
                    nc.vector.tensor_copy(st[:], ps[:])
                    nc.vector.tensor_add(st[:, 0:128], st[:, 0:128], st[:, 128:256])
                    nc.vector.tensor_add(st[:, 0:64], st[:, 0:64], st[:, 64:128])
                    nc.vector.tensor_add(st[:, 0:32], st[:, 0:32], st[:, 32:64])
                    nc.vector.tensor_add(
                        ob[l][:, i * C:(i + 1) * C], st[:, 0:16], st[:, 16:32]
                    )
            for l in range(3):
                nc.gpsimd.dma_start(o_out[l][:], ob[l][:])

    nc.finalize()
    # Walrus accepts at most 1 sync wait per engine instruction (2 on
    # InstEventSemaphore); Tile/bacc leave 2 on some DMACopies.  Move
    # excess waits onto EventSemaphore instructions inserted just before.
    for f in nc.m.functions:
        for blk in f.blocks:
            idx = 0
            while idx < len(blk.instructions):
                inst = blk.instructions[idx]
                si = getattr(inst, "sync_info", None)
                if (
                    si is not None
                    and si.on_wait
                    and len(si.on_wait) > 1
                    and not isinstance(inst, mybir.InstEventSemaphore)
                ):
                    moved, keep = si.on_wait[:-1], si.on_wait[-1:]
                    for wi in range(0, len(moved), 2):
                        ev = mybir.InstEventSemaphore(name=nc.get_next_instruction_name())
                        ev.engine = inst.engine
                        ev.sync_info = mybir.SyncInfo(
                            on_wait=list(moved[wi:wi + 2]), on_update=[]
                        )
                        nc.register_instruction(ev)
                        blk.instructions.insert(idx, ev)
                        idx += 1
                    si.on_wait = keep
                idx += 1
    _PROG = nc
    return nc


def _pack_w(wrel_l, l):
    """(A,A,tau,16) -> per-i SBUF image (A, 128, NCH*256), rows (j,q,c)."""
    w2 = np.ascontiguousarray(wrel_l, dtype=np.float32).reshape(A, NCH[l], 128, 256)
    return np.ascontiguousarray(w2.transpose(0, 2, 1, 3)).reshape(A, 128, NCH[l] * 256)


def _build_A(rep, Y, l):
    """Host lhsT: (A_i, 128, NCH*M) with rows (j,q,c), cols (b,z)."""
    aq = []
    for (l1, l2) in QL[l]:
        # A_q[b,i,j,z,c] = sum_{x,y} CG[x,y,z] rep[l1][b,i,x,c] Y[l2][b,i,j,y]
        aq.append(np.einsum("xyz,bixc,bijy->bijzc", _CG[(l1, l2, l)], rep[l1], Y[l2]))
    al = np.stack(aq, axis=3)  # (B,i,j,nq,Z,C)
    # -> (i, j, nq, C, B, Z) -> rows (j,q,c) x cols (b,z)
    al = np.ascontiguousarray(al.transpose(1, 2, 3, 5, 0, 4), dtype=np.float32)
    al = al.reshape(A, ROWS[l], ML[l])
    al = al.reshape(A, NCH[l], 128, ML[l]).transpose(0, 2, 1, 3)
    return np.ascontiguousarray(al).reshape(A, 128, NCH[l] * ML[l])


def kernel(**inputs):
    from concourse.bass_utils import run_bass_kernel_spmd

    _LAST_EXEC_NS.clear()

    v0 = np.asarray(inputs["v0"], dtype=np.float32)
    v1 = np.asarray(inputs["v1"], dtype=np.float32)
    v2 = np.asarray(inputs["v2"], dtype=np.float32)
    rel_pos = np.asarray(inputs["rel_pos"], dtype=np.float32)
    norms = np.asarray(inputs["norms"], dtype=np.float32)
    wnl = [[np.asarray(inputs[f"wnl{L}_{l}"], dtype=np.float32) for l in range(3)] for L in range(2)]
    wrel = [[np.asarray(inputs[f"wrel{L}_{l}"], dtype=np.float32) for l in range(3)] for L in range(2)]

    Y = _sph_np(rel_pos)
    conn = (norms < CUT).astype(np.float32)
    w_packs = [[_pack_w(wrel[L][l], l) for l in range(3)] for L in range(2)]

    nc = _build_program()
    parts = [v0, v1, v2]
    outs = []
    for L in range(LAYERS):
        mp = [np.einsum("bij,bjomc->biomc", conn, p) for p in parts]
        cg_nl = _cg_product_np(mp, mp)
        mixed_nl = [np.einsum("baomt,aotc->baomc", cg_nl[l], wnl[L][l]) for l in range(3)]
        rep = [mixed_nl[l][:, :, 0] for l in range(3)]  # (B,A,2l+1,C)
        a_packs = [_build_A(rep, Y, l) for l in range(3)]

        wa = [
            np.ascontiguousarray(np.concatenate([w_packs[L][l], a_packs[l]], axis=2))
            for l in range(3)
        ]
        in_maps = []
        for r in range(NCORES):
            sl = slice(r * IL, (r + 1) * IL)
            in_maps.append({f"wa{l}": np.ascontiguousarray(wa[l][sl]) for l in range(3)})

        res = run_bass_kernel_spmd(nc, in_maps, core_ids=list(range(NCORES)))
        if getattr(res, "exec_time_ns", None) is not None:
            _LAST_EXEC_NS.append(res.exec_time_ns)

        parts = []
        for l in range(3):
            po = np.zeros((B, A, 1, ZL[l], C), dtype=np.float32)
            for r in range(NCORES):
                o = res.results[r][f"o{l}"]  # (B*Z, IL*C)
                o = o.reshape(B, ZL[l], IL, C).transpose(0, 2, 1, 3)
                po[:, r * IL:(r + 1) * IL, 0] = o
            parts.append(po)
        outs.append(_scalars_np(parts))

    return np.concatenate(outs, axis=0)


# revision 17
# speedup vs baseline: 1.4386x; 1.0005x over previous
"""Trainium2 Bass kernel for nn_CGLayers (GElib-style CG message passing).

Strategy
--------
The only large data is the per-pair weights wrel{L}_{l}: 2 layers x
(32,32,tau_l,16) f32 = 503 MB total -> memory-bound stream.  Everything
else (CG tables, spherical harmonics Y, neighbor sums mp, cg_nl,
mixed_nl/rep) is tiny and depends on wrel only through the previous
layer's output, so it is computed on the host between two device
launches (one per layer).

Key algebraic fact: in cg_rel = cg_product(rep, sph) the sph channels
are all identical (reference repeats Y across C), so
  cg_rel[b,i,j,z,(q,c,d)] = A_q[b,i,j,z,c]           (d-independent)
  out[b,i,z,c'] = sum_{j,q,c} A_q[b,i,j,z,c] * (sum_d W[i,j,(q,c,d),c'])
The device therefore streams W with rows K=(j,q,c) and free axis
N=(d,c')=256 (contiguous 1KB blocks in the natural wrel layout), doing
PSUM-accumulated matmuls with the host-built A as the stationary
operand, then reduces over d (a 4-step halving add on the 256-wide
PSUM result).  Sharding: atom axis i (adim0) split 4-per-core across
8 cores; no collectives needed.
"""

import math
import sys

import numpy as np

sys.path.insert(0, "/opt/trn_rl_repo")

MAXL = 2
C = 16
A = 32
B = 2
LAYERS = 2
CUT = 0.5
NCORES = 8
IL = A // NCORES  # atoms per core

# couplings (l1,l2) per output l, in reference loop order (l1 outer, l2 inner)
QL = [
    [(0, 0), (1, 1), (2, 2)],
    [(0, 1), (1, 0), (1, 1), (1, 2), (2, 1), (2, 2)],
    [(0, 2), (1, 1), (1, 2), (2, 0), (2, 1), (2, 2)],
]
NQ = [len(q) for q in QL]          # 3, 6, 6
ZL = [1, 3, 5]                     # 2l+1
ML = [B * z for z in ZL]           # matmul M per output l: 2, 6, 10
ROWS = [A * nq * C for nq in NQ]   # K rows per i: 1536, 3072, 3072
NCH = [r // 128 for r in ROWS]     # 128-row chunks per i: 12, 24, 24


def _cg_coeff(l1, l2, l, m1, m2, m):
    if m1 + m2 != m or l < abs(l1 - l2) or l > l1 + l2:
        return 0.0
    f = math.factorial
    pre = math.sqrt((2 * l + 1) * f(l + l1 - l2) * f(l - l1 + l2) * f(l1 + l2 - l) / f(l1 + l2 + l + 1))
    pre *= math.sqrt(f(l + m) * f(l - m) * f(l1 + m1) * f(l1 - m1) * f(l2 + m2) * f(l2 - m2))
    s = 0.0
    for k in range(0, l1 + l2 + 1):
        d = (k, l1 + l2 - l - k, l1 - m1 - k, l2 + m2 - k, l - l2 + m1 + k, l - l1 - m2 + k)
        if min(d) < 0:
            continue
        s += (-1.0) ** k / (f(d[0]) * f(d[1]) * f(d[2]) * f(d[3]) * f(d[4]) * f(d[5]))
    return pre * s


def _cg_matrix(l1, l2, l):
    M = np.zeros((2 * l1 + 1, 2 * l2 + 1, 2 * l + 1), dtype=np.float32)
    for m1 in range(-l1, l1 + 1):
        for m2 in range(-l2, l2 + 1):
            m = m1 + m2
            if -l <= m <= l:
                M[m1 + l1, m2 + l2, m + l] = _cg_coeff(l1, l2, l, m1, m2, m)
    return M


_CG = {}
for _l1 in range(MAXL + 1):
    for _l2 in range(MAXL + 1):
        for _l in range(abs(_l1 - _l2), min(_l1 + _l2, MAXL) + 1):
            _CG[(_l1, _l2, _l)] = _cg_matrix(_l1, _l2, _l)


def _sph_np(rel_pos):
    """Real spherical harmonics of normalized rel vectors -> [(B,A,A,2l+1)]."""
    eps = np.float32(1e-6)
    r = np.sqrt(np.sum(rel_pos * rel_pos, axis=-1, keepdims=True) + eps)
    u = rel_pos / r
    x, y, z = u[..., 0], u[..., 1], u[..., 2]
    Y0 = np.float32(0.28209479) * np.ones_like(x)[..., None]
    Y1 = np.float32(0.48860251) * np.stack([y, z, x], axis=-1)
    Y2 = np.stack(
        [
            np.float32(1.09254843) * x * y,
            np.float32(1.09254843) * y * z,
            np.float32(0.31539157) * (np.float32(3.0) * z * z - np.float32(1.0)),
            np.float32(1.09254843) * x * z,
            np.float32(0.54627422) * (x * x - y * y),
        ],
        axis=-1,
    )
    return [Y0, Y1, Y2]


def _cg_product_np(parts1, parts2):
    out = [[] for _ in range(MAXL + 1)]
    for l1, p1 in enumerate(parts1):
        for l2, p2 in enumerate(parts2):
            for l in range(abs(l1 - l2), min(l1 + l2, MAXL) + 1):
                t = np.einsum("xyz,...xc,...yd->...zcd", _CG[(l1, l2, l)], p1, p2)
                out[l].append(t.reshape(t.shape[:-2] + (t.shape[-2] * t.shape[-1],)))
    return [np.concatenate(o, axis=-1) for o in out]


def _scalars_np(parts):
    b, a = parts[0].shape[0], parts[0].shape[1]
    s0 = parts[0].reshape(b, a, C)
    sn = [np.einsum("baomc,baomd->bacd", p, p).reshape(b, a, C * C) for p in parts]
    return np.concatenate([s0] + sn, axis=-1)


_PROG = None
_LAST_EXEC_NS = []


def _build_program():
    """One SPMD Bass program: per core, 4 atoms x 3 output-l weight streams."""
    global _PROG
    if _PROG is not None:
        return _PROG
    import concourse.bass as bass
    import concourse.mybir as mybir
    from concourse.tile import TileContext

    f32 = mybir.dt.float32
    nc = bass.Bass()
    wa_in, o_out = [], []
    for l in range(3):
        wa_in.append(
            nc.dram_tensor(f"wa{l}", [IL, 128, NCH[l] * (256 + ML[l])], f32, kind="ExternalInput")
        )
        o_out.append(nc.dram_tensor(f"o{l}", [ML[l], IL * C], f32, kind="ExternalOutput"))

    with TileContext(nc) as tc:
        with (
            tc.tile_pool(name="z", bufs=1) as zp,
            tc.tile_pool(name="w", bufs=2) as wp,
            tc.tile_pool(name="ps", bufs=2, space=bass.MemorySpace.PSUM) as pp,
            tc.tile_pool(name="s", bufs=2) as sp,
        ):
            # zero tile: feeds a dummy first matmul per group so the psum
            # buffer-reuse wait lands on a PE op with no other deps
            # (Matmult supports only a single sync wait on TRN2)
            z = zp.tile([16, 256], f32)
            nc.gpsimd.memset(z[:], 0.0)
            # per-l output staging: all (i,l) results land here via DVE,
            # then 3 contiguous DMAs at the end (keeps every inst <=1 wait)
            ob = [
                zp.tile([ML[l], IL * C], f32, tag=f"ob{l}", name=f"ob{l}")
                for l in range(3)
            ]
            for i in range(IL):
                for l in range(3):
                    m = ML[l]
                    woff = NCH[l] * 256
                    wt = wp.tile([128, NCH[l] * (256 + m)], f32, tag=f"w{l}")
                    nc.sync.dma_start(wt[:], wa_in[l][i])
                    ps = pp.tile([m, 256], f32, tag=f"ps{l}")
                    nc.tensor.matmul(ps[:], z[:, 0:m], z[:, 0:256], start=True, stop=False)
                    for k in range(NCH[l]):
                        nc.tensor.matmul(
                            ps[:],
                            wt[:, woff + k * m: woff + (k + 1) * m],
                            wt[:, k * 256:(k + 1) * 256],
                            start=False,
                            stop=(k == NCH[l] - 1),
                        )
                    # d-sum: free axis is (d,c') with d outer; halve d 4 times
                    st = sp.tile([m, 256], f32, tag=f"s{l}")
                    nc.vector.tensor_copy(st[:], ps[:])
                    nc.vector.tensor_add(st[:, 0:128], st[:, 0:128], st[:, 128:256])
                    nc.vector.tensor_add(st[:, 0:64], st[:, 0:64], st[:, 64:128])
                    nc.vector.tensor_add(st[:, 0:32], st[:, 0:32], st[:, 32:64])
                    nc.vector.tensor_add(
                        ob[l][:, i * C:(i + 1) * C], st[:, 0:16], st[:, 16:32]
                    )
            for l in range(3):
                nc.gpsimd.dma_start(o_out[l][:], ob[l][:])

    nc.finalize()
    # Walrus accepts at most 1 sync wait per engine instruction (2 on
    # InstEventSemaphore); Tile/bacc leave 2 on some DMACopies.  Move
    # excess waits onto EventSemaphore instructions inserted just before.
    for f in nc.m.functions:
        for blk in f.blocks:
            idx = 0
            while idx < len(blk.instructions):
                inst = blk.instructions[idx]
                si = getattr(inst, "sync_info", None)
                if (
                    si is not None
                    and si.on_wait
                    and len(si.on_wait) > 1
                    and not isinstance(inst, mybir.InstEventSemaphore)
                ):
                    moved, keep = si.on_wait[:-1], si.on_wait[-1:]
                    for wi in range(0, len(moved), 2):
                        ev = mybir.InstEventSemaphore(name=nc.get_next_instruction_name())
                        ev.engine = inst.engine
                        ev.sync_info = mybir.SyncInfo(
                            on_wait=list(moved[wi:wi + 2]), on_update=[]
                        )
                        nc.register_instruction(ev)
                        blk.instructions.insert(idx, ev)
                        idx += 1
                    si.on_wait = keep
                idx += 1
    _PROG = nc
    return nc


def _pack_w(wrel_l, l):
    """(A,A,tau,16) -> per-i SBUF image (A, 128, NCH*256), rows (j,q,c)."""
    w2 = np.ascontiguousarray(wrel_l, dtype=np.float32).reshape(A, NCH[l], 128, 256)
    return np.ascontiguousarray(w2.transpose(0, 2, 1, 3)).reshape(A, 128, NCH[l] * 256)


def _build_A(rep, Y, l):
    """Host lhsT: (A_i, 128, NCH*M) with rows (j,q,c), cols (b,z)."""
    aq = []
    for (l1, l2) in QL[l]:
        # A_q[b,i,j,z,c] = sum_{x,y} CG[x,y,z] rep[l1][b,i,x,c] Y[l2][b,i,j,y]
        aq.append(np.einsum("xyz,bixc,bijy->bijzc", _CG[(l1, l2, l)], rep[l1], Y[l2]))
    al = np.stack(aq, axis=3)  # (B,i,j,nq,Z,C)
    # -> (i, j, nq, C, B, Z) -> rows (j,q,c) x cols (b,z)
    al = np.ascontiguousarray(al.transpose(1, 2, 3, 5, 0, 4), dtype=np.float32)
    al = al.reshape(A, ROWS[l], ML[l])
    al = al.reshape(A, NCH[l], 128, ML[l]).transpose(0, 2, 1, 3)
    return np.ascontiguousarray(al).reshape(A, 128, NCH[l] * ML[l])


def kernel(**inputs):
    from concourse.bass_utils import run_bass_kernel_spmd

    _LAST_EXEC_NS.clear()

    v0 = np.asarray(inputs["v0"], dtype=np.float32)
    v1 = np.asarray(inputs["v1"], dtype=np.float32)
    v2 = np.asarray(inputs["v2"], dtype=np.float32)
    rel_pos = np.asarray(inputs["rel_pos"], dtype=np.float32)
    norms = np.asarray(inputs["norms"], dtype=np.float32)
    wnl = [[np.asarray(inputs[f"wnl{L}_{l}"], dtype=np.float32) for l in range(3)] for L in range(2)]
    wrel = [[np.asarray(inputs[f"wrel{L}_{l}"], dtype=np.float32) for l in range(3)] for L in range(2)]

    Y = _sph_np(rel_pos)
    conn = (norms < CUT).astype(np.float32)
    w_packs = [[_pack_w(wrel[L][l], l) for l in range(3)] for L in range(2)]

    nc = _build_program()
    parts = [v0, v1, v2]
    outs = []
    for L in range(LAYERS):
        mp = [np.einsum("bij,bjomc->biomc", conn, p) for p in parts]
        cg_nl = _cg_product_np(mp, mp)
        mixed_nl = [np.einsum("baomt,aotc->baomc", cg_nl[l], wnl[L][l]) for l in range(3)]
        rep = [mixed_nl[l][:, :, 0] for l in range(3)]  # (B,A,2l+1,C)
        a_packs = [_build_A(rep, Y, l) for l in range(3)]

        wa = [
            np.ascontiguousarray(np.concatenate([w_packs[L][l], a_packs[l]], axis=2))
            for l in range(3)
        ]
        in_maps = []
        for r in range(NCORES):
            sl = slice(r * IL, (r + 1) * IL)
            in_maps.append({f"wa{l}": np.ascontiguousarray(wa[l][sl]) for l in range(3)})

        res = run_bass_kernel_spmd(nc, in_maps, core_ids=list(range(NCORES)))
        if getattr(res, "exec_time_ns", None) is not None:
            _LAST_EXEC_NS.append(res.exec_time_ns)

        parts = []
        for l in range(3):
            po = np.zeros((B, A, 1, ZL[l], C), dtype=np.float32)
            for r in range(NCORES):
                o = res.results[r][f"o{l}"]  # (B*Z, IL*C)
                o = o.reshape(B, ZL[l], IL, C).transpose(0, 2, 1, 3)
                po[:, r * IL:(r + 1) * IL, 0] = o
            parts.append(po)
        outs.append(_scalars_np(parts))

    return np.concatenate(outs, axis=0)
